# revision 1
# baseline (speedup 1.0000x reference)
"""Trainium2 Bass kernel for ViTDet-style attention with decomposed
relative-position bias.

Problem shapes (hardcoded):
  x: (4, 32, 32, 768) f32, Wqkv: (768, 2304), Wproj: (768, 768),
  bproj: (768,), rel_pos_h/w: (63, 64).
  12 heads, head_dim 64, S = 32*32 = 1024.

Sharding: 48 (batch, head) pairs -> 6 heads per core, all of one batch per
core-pair. Each core computes its heads' attention and a partial output
projection (its heads' channel rows of Wproj); the host sums the two
partials per batch and adds bproj.

Device algorithm per core (bf16 matmuls, fp32 PSUM accumulation):
  - qkT = Wqk^T @ x^T  (x^T supplied pre-transposed by host; k pre-scaled)
  - v   = x @ Wv       (natural layout, with an appended ones column)
  - PhT = rel_pos_h^T @ qT; band-extract BhT[kh',(h,w)] = PhT[kh'+h,(h,w)]
    on the PE via shifted-identity selection matmuls (same for W axis)
  - scoresT (k x q) = kaugT^T @ qaugT in ONE K=128 matmul per tile:
    aug rows 0-63 = kT / qT, 64-95 = one-hot(h) / BhT, 96-127 = one-hot(w)/BwT
    => rel-pos bias folded into the QK matmul for free.
  - eT = exp(scoresT) on ScalarE (no max subtraction; scores are O(1)).
  - avT (65 x q) accumulates v_aug^T-matmul over k blocks; row 64 = softmax
    denominator via the ones column.
  - normalize via DVE reciprocal + gpsimd partition-broadcast + DVE multiply.
  - partial = out_heads @ Wproj_shard  (natural layout, DMA PSUM->DRAM).
"""

import numpy as np

import concourse.bass as bass
import concourse.bacc as bacc
import concourse.mybir as mybir
import concourse.tile as tile
from concourse.bass_utils import run_bass_kernel_spmd

F32 = mybir.dt.float32
F32R = mybir.dt.float32r
BF16 = mybir.dt.bfloat16

NH = 12          # total heads
C = 768
HD = 64
H = W = 32
S = H * W        # 1024
B = 4
NCORES = 8
HPC = NH * B // NCORES   # heads per core = 6
NCH = 6                  # C // 128 input-channel chunks
NKB = S // 128           # 8 k blocks
NQB = S // 128           # 8 q blocks
NHALF = 512              # matmul moving-dim half


def _r(ap):
    # operands are already float32r-typed
    return ap


def build_program():
    nc = bacc.Bacc("TRN2", target_bir_lowering=False, debug=False)

    xT = nc.declare_dram_parameter("xT", [C, S], BF16, isOutput=False)
    wqk = nc.declare_dram_parameter("wqk", [C, 2 * HPC * HD], BF16, isOutput=False)
    wv = nc.declare_dram_parameter("wv", [C, HPC * HD], BF16, isOutput=False)
    wproj = nc.declare_dram_parameter("wproj", [HPC * HD, C], BF16, isOutput=False)
    rhT = nc.declare_dram_parameter("rhT", [HD, 2 * H - 1], BF16, isOutput=False)
    rwT = nc.declare_dram_parameter("rwT", [HD, 2 * W - 1], BF16, isOutput=False)
    onehot = nc.declare_dram_parameter("onehot", [65, S], BF16, isOutput=False)
    idband = nc.declare_dram_parameter("idband", [2 * H - 1, 3 * W - 1], BF16,
                                       isOutput=False)
    out = nc.declare_dram_parameter("out", [S, C], F32, isOutput=True)

    # small DRAM bounce buffers for the rowsum transpose (I/O tensors --
    # internal DRAM scratch is paged and much slower for strided DMAs)
    rs_dram = nc.declare_dram_parameter("rs_dram", [S], F32, isOutput=True)
    rc_dram = nc.declare_dram_parameter("rc_dram", [S], F32, isOutput=True)

    with tile.TileContext(nc) as tc:
        with (
            tc.tile_pool(name="persist", bufs=1) as persist,
            tc.tile_pool(name="psum_big", bufs=2, space="PSUM") as psum_big,
            tc.tile_pool(name="psum_av", bufs=2, space="PSUM") as psum_av,
            tc.tile_pool(name="et", bufs=3) as et_pool,
            tc.tile_pool(name="small", bufs=2) as small,
        ):
            # ---- persistent SBUF loads ----
            xT_sb = []
            for ci in range(NCH):
                t = persist.tile([128, S], BF16, tag=f"xT{ci}", name=f"xT{ci}")
                nc.sync.dma_start(t[:], xT[128 * ci:128 * (ci + 1), :])
                xT_sb.append(t)
            wqk_sb = []
            for ci in range(NCH):
                t = persist.tile([128, 2 * HPC * HD], BF16, tag=f"wqk{ci}", name=f"wqk{ci}")
                nc.sync.dma_start(t[:], wqk[128 * ci:128 * (ci + 1), :])
                wqk_sb.append(t)
            wv_sb = []
            for ci in range(NCH):
                t = persist.tile([128, HPC * HD], BF16, tag=f"wv{ci}", name=f"wv{ci}")
                nc.sync.dma_start(t[:], wv[128 * ci:128 * (ci + 1), :])
                wv_sb.append(t)
            wproj_sb = []
            for ci in range(HPC * HD // 128):
                t = persist.tile([128, C], BF16, tag=f"wproj{ci}", name=f"wproj{ci}")
                nc.sync.dma_start(t[:], wproj[128 * ci:128 * (ci + 1), :])
                wproj_sb.append(t)
            idb_sb = persist.tile([2 * H - 1, 3 * W - 1], BF16, tag="idb",
                                  name="idb_sb")
            nc.sync.dma_start(idb_sb[:], idband[:, :])
            rhT_sb = persist.tile([HD, 2 * H - 1], BF16, tag="rhT", name="rhT_sb")
            nc.sync.dma_start(rhT_sb[:], rhT[:, :])
            rwT_sb = persist.tile([HD, 2 * W - 1], BF16, tag="rwT", name="rwT_sb")
            nc.sync.dma_start(rwT_sb[:], rwT[:, :])

            # ---- one-hot template (65, S), host-supplied constant ----
            # rows 0-63: one-hot(h)/one-hot(w) reversed; row 64: all ones
            oh = persist.tile([65, S], BF16, tag="onehot", name="onehot")
            nc.sync.dma_start(oh[:], onehot[:, :])

            # ---- augmented k/q tiles (128, S) per head ----
            kaug = [persist.tile([128, S], BF16, tag=f"kaug{i}", name=f"kaug{i}") for i in range(HPC)]
            qaug = [persist.tile([128, S], BF16, tag=f"qaug{i}", name=f"qaug{i}") for i in range(HPC)]
            for i in range(HPC):
                nc.vector.tensor_copy(kaug[i][64:128, :], oh[0:64, :])

            # ---- v projection (natural) + ones column ----
            # v_sb[sb]: (128, 6*65) cols [65i..65i+64) = head i v, col 65i+64 = 1
            v_sb = [persist.tile([128, HPC * (HD + 1)], BF16, tag=f"v{sb}", name=f"v{sb}")
                    for sb in range(NKB)]
            for sb in range(NKB):
                vp = psum_big.tile([128, HPC * HD + HPC], F32, tag="big", name="vp")
                for ci in range(NCH):
                    nc.tensor.matmul(
                        vp[:, 0:HPC * HD],
                        _r(xT_sb[ci][:, 128 * sb:128 * (sb + 1)]),
                        _r(wv_sb[ci][:]),
                        start=(ci == 0), stop=(ci == NCH - 1))
                # ones columns via outer product of the ones row
                nc.tensor.matmul(vp[:, HPC * HD:HPC * HD + HPC],
                                 oh[64:65, 128 * sb:128 * (sb + 1)],
                                 oh[64:65, 0:HPC], start=True, stop=True)
                src = bass.AP(vp.tensor, vp[:].offset,
                              [vp[:].ap[0], [HD, HPC], [1, HD]])
                dst = bass.AP(v_sb[sb].tensor, v_sb[sb][:].offset,
                              [v_sb[sb][:].ap[0], [HD + 1, HPC], [1, HD]])
                nc.vector.tensor_copy(dst, src)
                ones_src = bass.AP(vp.tensor, vp[:].offset + HPC * HD,
                                   [vp[:].ap[0], [1, HPC]])
                ones_dst = bass.AP(v_sb[sb].tensor, v_sb[sb][:].offset + HD,
                                   [v_sb[sb][:].ap[0], [HD + 1, HPC]])
                nc.vector.tensor_copy(ones_dst, ones_src)

            # ---- qk projection (transposed layout) ----
            # qkT octile t covers oc rows [128t, 128t+128): t<3 -> q, t>=3 -> k
            for t in range(2 * HPC * HD // 128):
                qp = psum_big.tile([128, S], F32, tag="big", name="qp")
                for ci in range(NCH):
                    for nh in range(S // NHALF):
                        nc.tensor.matmul(
                            qp[:, NHALF * nh:NHALF * (nh + 1)],
                            _r(wqk_sb[ci][:, 128 * t:128 * (t + 1)]),
                            _r(xT_sb[ci][:, NHALF * nh:NHALF * (nh + 1)]),
                            start=(ci == 0), stop=(ci == NCH - 1))
                for sub in range(2):
                    head = (t % 3) * 2 + sub
                    dst = (qaug if t < 3 else kaug)[head]
                    if t < 3:
                        nc.scalar.copy(dst[0:64, :], qp[64 * sub:64 * sub + 64, :])
                    else:
                        nc.vector.tensor_copy(dst[0:64, :],
                                              qp[64 * sub:64 * sub + 64, :])

            # ---- per head: rel-pos tables -> band-gather into qaug ----
            for i in range(HPC):
                php = psum_big.tile([2 * H - 1, S], F32, tag="big", name="php")
                pwp = psum_big.tile([2 * W - 1, S], F32, tag="big", name="pwp")
                for nh in range(S // NHALF):
                    sl = slice(NHALF * nh, NHALF * (nh + 1))
                    nc.tensor.matmul(php[:, sl], _r(rhT_sb[:]),
                                     _r(qaug[i][0:64, sl]), start=True, stop=True)
                    nc.tensor.matmul(pwp[:, sl], _r(rwT_sb[:]),
                                     _r(qaug[i][0:64, sl]), start=True, stop=True)
                ph_sb = small.tile([2 * H - 1, S], BF16, tag="ph_sb",
                                   name="ph_sb", bufs=2)
                pw_sb = small.tile([2 * W - 1, S], BF16, tag="pw_sb",
                                   name="pw_sb", bufs=2)
                nc.scalar.copy(ph_sb[:], php[:])
                nc.vector.tensor_copy(pw_sb[:], pwp[:])
                # band-extract on PE: BhT_rev[kh', (h,w)] = PhT[kh'+h, (h,w)]
                # = sum_r idband[r, kh'+h] * PhT[r, (h,w)]  (idband = I_63)
                bhp = psum_big.tile([H, S], F32, tag="big", name="bhp")
                bwp = psum_big.tile([W, S], F32, tag="big", name="bwp")
                for h in range(H):
                    nc.tensor.matmul(bhp[:, W * h:W * (h + 1)],
                                     idb_sb[:, h:h + H],
                                     ph_sb[:, W * h:W * (h + 1)],
                                     start=True, stop=True)
                for w in range(W):
                    # w-major output block: bwp[kw', w*32+h] = PwT[kw'+w,(h,w)]
                    rhs_w = bass.AP(pw_sb.tensor, pw_sb[:].offset + w,
                                    [pw_sb[:].ap[0], [W, H]])
                    nc.tensor.matmul(bwp[:, H * w:H * (w + 1)],
                                     idb_sb[:, w:w + W], rhs_w,
                                     start=True, stop=True)
                nc.scalar.copy(qaug[i][64:96, :], bhp[:])
                # permute w-major back to (h, w) order during the copy
                bwp_perm = bass.AP(bwp.tensor, bwp[:].offset,
                                   [bwp[:].ap[0], [1, H], [H, W]])
                nc.vector.tensor_copy(qaug[i][96:128, :], bwp_perm)

            # ---- attention per head ----
            out_headsT = [persist.tile([128, S], BF16, tag=f"ohT{c}",
                                       name=f"ohT{c}")
                          for c in range(HPC * HD // 128)]
            for i in range(HPC):
                av = psum_av.tile([HD + 1, S], F32, tag="av", name="av")
                for kb in range(NKB):
                    sc = psum_big.tile([128, S], F32, tag="big", name="qp")
                    for nh in range(S // NHALF):
                        sl = slice(NHALF * nh, NHALF * (nh + 1))
                        nc.tensor.matmul(
                            sc[:, sl],
                            _r(kaug[i][:, 128 * kb:128 * (kb + 1)]),
                            _r(qaug[i][:, sl]), start=True, stop=True)
                    e = et_pool.tile([128, S], BF16, tag="et", name="et")
                    nc.scalar.activation(e[:], sc[:],
                                         mybir.ActivationFunctionType.Exp)
                    for nh in range(S // NHALF):
                        sl = slice(NHALF * nh, NHALF * (nh + 1))
                        nc.tensor.matmul(
                            av[:, sl],
                            _r(v_sb[kb][:, (HD + 1) * i:(HD + 1) * (i + 1)]),
                            _r(e[:, sl]),
                            start=(kb == 0), stop=(kb == NKB - 1))
                rowsum = small.tile([1, S], F32, tag="rowsum", name="rowsum",
                                    bufs=1)
                nc.scalar.copy(rowsum[:], av[HD:HD + 1, :])
                nc.sync.dma_start(bass.AP(rs_dram, 0, [[1, S]]), rowsum[:])
                rs_t = small.tile([128, NQB], F32, tag="rs_t", name="rs_t")
                nc.sync.dma_start(
                    rs_t[:], bass.AP(rs_dram, 0, [[1, 128], [128, NQB]]))
                rc_t = small.tile([128, NQB], F32, tag="rc_t", name="rc_t")
                nc.vector.reciprocal(rc_t[:], rs_t[:])
                nc.sync.dma_start(
                    bass.AP(rc_dram, 0, [[1, 128], [128, NQB]]), rc_t[:])
                recip = small.tile([1, S], F32, tag="recip", name="recip",
                                   bufs=1)
                nc.sync.dma_start(recip[:], bass.AP(rc_dram, 0, [[1, S]]))
                rb = small.tile([64, S], F32, tag="rbcast", name="rbcast",
                                bufs=1)
                nc.gpsimd.partition_broadcast(rb[:], recip[:])
                chunk, row = i // 2, (i % 2) * 64
                nc.vector.tensor_tensor(
                    out_headsT[chunk][row:row + 64, :], av[0:HD, :], rb[:],
                    op=mybir.AluOpType.mult)

            # ---- output projection (partial) ----
            for qb in range(NQB):
                pp = psum_big.tile([128, C], F32, tag="big", name="pp")
                for ci in range(HPC * HD // 128):
                    nc.tensor.matmul(
                        pp[:, 0:NHALF],
                        _r(out_headsT[ci][:, 128 * qb:128 * (qb + 1)]),
                        _r(wproj_sb[ci][:, 0:NHALF]),
                        start=(ci == 0), stop=(ci == 2))
                    nc.tensor.matmul(
                        pp[:, NHALF:C],
                        _r(out_headsT[ci][:, 128 * qb:128 * (qb + 1)]),
                        _r(wproj_sb[ci][:, NHALF:C]),
                        start=(ci == 0), stop=(ci == 2))
                pp_sb = small.tile([128, C], F32, tag="pp_sb", name="pp_sb", bufs=1)
                (nc.scalar.copy if qb % 2 else nc.vector.tensor_copy)(
                    pp_sb[:], pp[:])
                nc.sync.dma_start(out[128 * qb:128 * (qb + 1), :], pp_sb[:])

    nc.compile()
    return nc


def shard_inputs(x, Wqkv, Wproj, rel_pos_h, rel_pos_w):
    """Build the 8 per-core input maps."""
    import ml_dtypes
    bf16 = ml_dtypes.bfloat16
    scale = HD ** (-0.5)
    x = np.asarray(x, dtype=np.float32)
    Wqkv = np.asarray(Wqkv, dtype=np.float32)
    Wproj = np.asarray(Wproj, dtype=np.float32)
    rhT = np.ascontiguousarray(np.asarray(rel_pos_h, np.float32).T).astype(bf16)
    rwT = np.ascontiguousarray(np.asarray(rel_pos_w, np.float32).T).astype(bf16)
    idb = np.zeros((2 * H - 1, 3 * W - 1), np.float32)
    for r in range(2 * H - 1):
        idb[r, r] = 1.0
    idb = idb.astype(bf16)
    oh = np.zeros((65, S), np.float32)
    for khp in range(H):
        oh[khp, (31 - khp) * W:(31 - khp) * W + W] = 1.0
    for kwp in range(W):
        oh[32 + kwp, 31 - kwp::W] = 1.0
    oh[64, :] = 1.0
    oh = oh.astype(bf16)
    in_maps = []
    for core in range(NCORES):
        b = core // 2
        h0 = (core % 2) * HPC
        xb = x[b].reshape(S, C)
        xT = np.ascontiguousarray(xb.T).astype(bf16)
        wq = Wqkv[:, h0 * HD:(h0 + HPC) * HD]
        wk = Wqkv[:, C + h0 * HD:C + (h0 + HPC) * HD] * scale
        wqk = np.ascontiguousarray(np.concatenate([wq, wk], axis=1)).astype(bf16)
        wv = np.ascontiguousarray(
            Wqkv[:, 2 * C + h0 * HD:2 * C + (h0 + HPC) * HD]).astype(bf16)
        wp = np.ascontiguousarray(Wproj[h0 * HD:(h0 + HPC) * HD, :]).astype(bf16)
        in_maps.append({"xT": xT, "wqk": wqk, "wv": wv, "wproj": wp,
                        "rhT": rhT, "rwT": rwT, "onehot": oh,
                        "idband": idb})
    return in_maps


_NC_CACHE = {}


def kernel(x, Wqkv, Wproj, bproj, rel_pos_h, rel_pos_w):
    if "nc" not in _NC_CACHE:
        _NC_CACHE["nc"] = build_program()
    nc = _NC_CACHE["nc"]
    in_maps = shard_inputs(x, Wqkv, Wproj, rel_pos_h, rel_pos_w)
    res = run_bass_kernel_spmd(nc, in_maps, list(range(NCORES)))
    bproj = np.asarray(bproj, dtype=np.float32)
    out = np.empty((B, H, W, C), dtype=np.float32)
    for b in range(B):
        acc = res.results[2 * b]["out"] + res.results[2 * b + 1]["out"] + bproj
        out[b] = acc.reshape(H, W, C)
    return out



# revision 7
# speedup vs baseline: 1.3104x; 1.3104x over previous
"""Trainium2 Bass kernel for ViTDet-style attention with decomposed
relative-position bias.

Problem shapes (hardcoded):
  x: (4, 32, 32, 768) f32, Wqkv: (768, 2304), Wproj: (768, 768),
  bproj: (768,), rel_pos_h/w: (63, 64).
  12 heads, head_dim 64, S = 32*32 = 1024.

Sharding: 48 (batch, head) pairs -> 6 heads per core, all of one batch per
core-pair. Each core computes its heads' attention and a partial output
projection (its heads' channel rows of Wproj); the host sums the two
partials per batch and adds bproj.

Device algorithm per core (bf16 matmuls, fp32 PSUM accumulation):
  - qkT = Wqk^T @ x^T  (x^T supplied pre-transposed by host; k pre-scaled)
  - v   = x @ Wv       (natural layout); v_sb per-head segment is
    [v_i (64 cols) | ones (64 cols)] so the av matmul also produces the
    softmax denominator replicated on PSUM partitions 64-127.
  - rel-pos bias computed DIRECTLY in band form (no intermediate table
    product): BhT[r, (h,w)] = sum_c rhT[c, h+r] qT[c, (h,w)] via windowed
    stationaries; two h-values packed per 64x64 matmul (diagonal blocks
    used, off-diagonal garbage ignored).
  - scoresT (k x q) = kaugT^T @ qaugT in ONE K=128 matmul per tile:
    aug rows 0-63 = kT / qT, 64-95 = one-hot(h) / BhT, 96-127 = one-hot(w)/BwT
    => rel-pos bias folded into the QK matmul for free.
  - eT = exp(scoresT) on ScalarE (no max subtraction; scores are O(1)).
  - avT (128 x q): rows 0-63 = out accum, rows 64-127 = denominator.
  - normalize: DVE reciprocal of av[64:128] + DVE multiply. No DMA bounce.
  - partial = out_heads @ Wproj_shard  (natural layout, DMA PSUM->DRAM).
"""

import numpy as np

import concourse.bass as bass
import concourse.bacc as bacc
import concourse.mybir as mybir
import concourse.tile as tile
from concourse.bass_utils import run_bass_kernel_spmd

F32 = mybir.dt.float32
BF16 = mybir.dt.bfloat16

NH = 12          # total heads
C = 768
HD = 64
H = W = 32
S = H * W        # 1024
B = 4
NCORES = 8
HPC = NH * B // NCORES   # heads per core = 6
NCH = 6                  # C // 128 input-channel chunks
NKB = S // 128           # 8 k blocks
NQB = S // 128           # 8 q blocks
NHALF = 512              # matmul moving-dim half


def build_program():
    nc = bacc.Bacc("TRN2", target_bir_lowering=False, debug=False)

    xT = nc.declare_dram_parameter("xT", [C, S], BF16, isOutput=False)
    wqk = nc.declare_dram_parameter("wqk", [C, 2 * HPC * HD], BF16, isOutput=False)
    wv = nc.declare_dram_parameter("wv", [C, HPC * HD], BF16, isOutput=False)
    wproj = nc.declare_dram_parameter("wproj", [HPC * HD, C], BF16, isOutput=False)
    # windowed rel-pos tables: win[:, 64p+32j+r] = T[:, 2p+j+r]
    rh_win = nc.declare_dram_parameter("rh_win", [HD, S], BF16, isOutput=False)
    rw_win = nc.declare_dram_parameter("rw_win", [HD, S], BF16, isOutput=False)
    onehot = nc.declare_dram_parameter("onehot", [64, S], BF16, isOutput=False)
    out = nc.declare_dram_parameter("out", [S, C], F32, isOutput=True)

    with tile.TileContext(nc) as tc:
        with (
            tc.tile_pool(name="persist", bufs=1) as persist,
            tc.tile_pool(name="ps_sc", bufs=2, space="PSUM") as ps_sc,
            tc.tile_pool(name="ps_aux", bufs=2, space="PSUM") as ps_aux,
            tc.tile_pool(name="et", bufs=3) as et_pool,
            tc.tile_pool(name="small", bufs=2) as small,
        ):
            # ---- persistent SBUF loads (interleaved so ci=0 compute can
            # start while later chunks stream in) ----
            xT_sb, wqk_sb, wv_sb = [], [], []
            for ci in range(NCH):
                t = persist.tile([128, S], BF16, tag=f"xT{ci}", name=f"xT{ci}")
                nc.sync.dma_start(t[:], xT[128 * ci:128 * (ci + 1), :])
                xT_sb.append(t)
                t = persist.tile([128, 2 * HPC * HD], BF16, tag=f"wqk{ci}",
                                 name=f"wqk{ci}")
                nc.sync.dma_start(t[:], wqk[128 * ci:128 * (ci + 1), :])
                wqk_sb.append(t)
                t = persist.tile([128, HPC * HD], BF16, tag=f"wv{ci}",
                                 name=f"wv{ci}")
                nc.sync.dma_start(t[:], wv[128 * ci:128 * (ci + 1), :])
                wv_sb.append(t)
            wproj_sb = []
            for ci in range(HPC * HD // 128):
                t = persist.tile([128, C], BF16, tag=f"wproj{ci}", name=f"wproj{ci}")
                nc.sync.dma_start(t[:], wproj[128 * ci:128 * (ci + 1), :])
                wproj_sb.append(t)
            rh_sb = persist.tile([HD, S], BF16, tag="rh", name="rh_sb")
            nc.sync.dma_start(rh_sb[:], rh_win[:, :])
            rw_sb = persist.tile([HD, S], BF16, tag="rw", name="rw_sb")
            nc.sync.dma_start(rw_sb[:], rw_win[:, :])

            # ---- one-hot template (64, S), host-supplied constant ----
            oh = persist.tile([64, S], BF16, tag="onehot", name="onehot")
            nc.sync.dma_start(oh[:], onehot[:, :])

            # ---- augmented k/q tiles (128, S) per head ----
            kaug = [persist.tile([128, S], BF16, tag=f"kaug{i}", name=f"kaug{i}")
                    for i in range(HPC)]
            qaug = [persist.tile([128, S], BF16, tag=f"qaug{i}", name=f"qaug{i}")
                    for i in range(HPC)]
            for i in range(HPC):
                nc.vector.tensor_copy(kaug[i][64:128, :], oh[:, :])

            # ---- v tiles: per-head segment [v_i (64) | ones (64)] ----
            v_sb = [persist.tile([128, HPC * 2 * HD], BF16, tag=f"v{sb}",
                                 name=f"v{sb}")
                    for sb in range(NKB)]
            for sb in range(NKB):
                ones_dst = bass.AP(v_sb[sb].tensor, v_sb[sb][:].offset + HD,
                                   [v_sb[sb][:].ap[0], [2 * HD, HPC], [1, HD]])
                nc.gpsimd.memset(ones_dst, 1.0)

            # ---- v projection (natural) ----
            for sb in range(NKB):
                vp = ps_aux.tile([128, S], F32, tag="aux", name="vp")
                for ci in range(NCH):
                    nc.tensor.matmul(
                        vp[:, 0:HPC * HD],
                        xT_sb[ci][:, 128 * sb:128 * (sb + 1)],
                        wv_sb[ci][:],
                        start=(ci == 0), stop=(ci == NCH - 1))
                src = bass.AP(vp.tensor, vp[:].offset,
                              [vp[:].ap[0], [HD, HPC], [1, HD]])
                dst = bass.AP(v_sb[sb].tensor, v_sb[sb][:].offset,
                              [v_sb[sb][:].ap[0], [2 * HD, HPC], [1, HD]])
                nc.vector.tensor_copy(dst, src)

            # ---- qk projection (transposed layout) ----
            # qkT octile t covers oc rows [128t, 128t+128): t<3 -> q, t>=3 -> k
            for t in range(2 * HPC * HD // 128):
                qp = ps_aux.tile([128, S], F32, tag="aux", name="qp")
                for ci in range(NCH):
                    for nh in range(S // NHALF):
                        nc.tensor.matmul(
                            qp[:, NHALF * nh:NHALF * (nh + 1)],
                            wqk_sb[ci][:, 128 * t:128 * (t + 1)],
                            xT_sb[ci][:, NHALF * nh:NHALF * (nh + 1)],
                            start=(ci == 0), stop=(ci == NCH - 1))
                for sub in range(2):
                    head = (t % 3) * 2 + sub
                    dst = (qaug if t < 3 else kaug)[head]
                    nc.scalar.copy(dst[0:64, :], qp[64 * sub:64 * sub + 64, :])

            # ---- per head: direct banded rel-pos bias into qaug rows 64-127
            # BhT[r, (h,w)] = sum_c rhT[c, h+r] qT[c, (h,w)]  (r, h in [0,32))
            # Two h-values per matmul: stationary (64, 64) = two overlapping
            # 32-col windows of the table; useful output = diagonal blocks.
            for i in range(HPC):
                bh = ps_sc.tile([64, S], F32, tag="sc", name="bh")
                bw = ps_sc.tile([64, S], F32, tag="sc", name="bw")
                for p in range(16):
                    nc.tensor.matmul(bh[:, 64 * p:64 * (p + 1)],
                                     rh_sb[:, 64 * p:64 * (p + 1)],
                                     qaug[i][0:64, 64 * p:64 * (p + 1)],
                                     start=True, stop=True)
                for p in range(16):
                    rw = bass.AP(qaug[i].tensor, qaug[i][:].offset + 2 * p,
                                 [[S, 64], [1, 2], [W, H]])
                    nc.tensor.matmul(bw[:, 64 * p:64 * (p + 1)],
                                     rw_sb[:, 64 * p:64 * (p + 1)],
                                     rw, start=True, stop=True)
                # copies: diagonal blocks -> qaug rows 64-127 (vector engine)
                pitch = bh[:].ap[0][0]
                # h-axis, j=0: h=2p
                nc.vector.tensor_copy(
                    bass.AP(qaug[i].tensor, qaug[i][:].offset + 64 * S,
                            [[S, 32], [64, 16], [1, 32]]),
                    bass.AP(bh.tensor, bh[:].offset,
                            [[pitch, 32], [64, 16], [1, 32]]))
                # h-axis, j=1: h=2p+1
                nc.vector.tensor_copy(
                    bass.AP(qaug[i].tensor, qaug[i][:].offset + 64 * S + 32,
                            [[S, 32], [64, 16], [1, 32]]),
                    bass.AP(bh.tensor, bh[:].offset + 32 * pitch + 32,
                            [[pitch, 32], [64, 16], [1, 32]]))
                # w-axis, j=0: w=2p, dst col = 32h + w
                nc.vector.tensor_copy(
                    bass.AP(qaug[i].tensor, qaug[i][:].offset + 96 * S,
                            [[S, 32], [2, 16], [W, H]]),
                    bass.AP(bw.tensor, bw[:].offset,
                            [[pitch, 32], [64, 16], [1, 32]]))
                # w-axis, j=1: w=2p+1
                nc.vector.tensor_copy(
                    bass.AP(qaug[i].tensor, qaug[i][:].offset + 96 * S + 1,
                            [[S, 32], [2, 16], [W, H]]),
                    bass.AP(bw.tensor, bw[:].offset + 32 * pitch + 32,
                            [[pitch, 32], [64, 16], [1, 32]]))

            # ---- attention per head ----
            out_headsT = [persist.tile([128, S], BF16, tag=f"ohT{c}",
                                       name=f"ohT{c}")
                          for c in range(HPC * HD // 128)]
            for i in range(HPC):
                av = ps_aux.tile([128, S], F32, tag="aux", name="av")
                for kb in range(NKB):
                    sc = ps_sc.tile([128, S], F32, tag="sc", name="sc")
                    for nh in range(S // NHALF):
                        sl = slice(NHALF * nh, NHALF * (nh + 1))
                        nc.tensor.matmul(
                            sc[:, sl],
                            kaug[i][:, 128 * kb:128 * (kb + 1)],
                            qaug[i][:, sl], start=True, stop=True)
                    e = et_pool.tile([128, S], BF16, tag="et", name="et")
                    nc.scalar.activation(e[:], sc[:],
                                         mybir.ActivationFunctionType.Exp)
                    for nh in range(S // NHALF):
                        sl = slice(NHALF * nh, NHALF * (nh + 1))
                        nc.tensor.matmul(
                            av[:, sl],
                            v_sb[kb][:, 2 * HD * i:2 * HD * (i + 1)],
                            e[:, sl],
                            start=(kb == 0), stop=(kb == NKB - 1))
                rb = small.tile([64, S], F32, tag="rb", name="rb")
                nc.vector.reciprocal(rb[:], av[64:128, :])
                chunk, row = i // 2, (i % 2) * 64
                nc.vector.tensor_tensor(
                    out_headsT[chunk][row:row + 64, :], av[0:HD, :], rb[:],
                    op=mybir.AluOpType.mult)

            # ---- output projection (partial) ----
            for qb in range(NQB):
                pp = ps_aux.tile([128, S], F32, tag="aux", name="pp")
                for ci in range(HPC * HD // 128):
                    nc.tensor.matmul(
                        pp[:, 0:NHALF],
                        out_headsT[ci][:, 128 * qb:128 * (qb + 1)],
                        wproj_sb[ci][:, 0:NHALF],
                        start=(ci == 0), stop=(ci == 2))
                    nc.tensor.matmul(
                        pp[:, NHALF:C],
                        out_headsT[ci][:, 128 * qb:128 * (qb + 1)],
                        wproj_sb[ci][:, NHALF:C],
                        start=(ci == 0), stop=(ci == 2))
                pp_sb = small.tile([128, C], F32, tag="pp_sb", name="pp_sb")
                (nc.scalar.copy if qb % 2 else nc.vector.tensor_copy)(
                    pp_sb[:], pp[:, 0:C])
                nc.sync.dma_start(out[128 * qb:128 * (qb + 1), :], pp_sb[:])

    nc.compile()
    return nc


def shard_inputs(x, Wqkv, Wproj, rel_pos_h, rel_pos_w):
    """Build the 8 per-core input maps."""
    import ml_dtypes
    bf16 = ml_dtypes.bfloat16
    scale = HD ** (-0.5)
    x = np.asarray(x, dtype=np.float32)
    Wqkv = np.asarray(Wqkv, dtype=np.float32)
    Wproj = np.asarray(Wproj, dtype=np.float32)
    rhT = np.ascontiguousarray(np.asarray(rel_pos_h, np.float32).T)
    rwT = np.ascontiguousarray(np.asarray(rel_pos_w, np.float32).T)

    def windowed(T):
        win = np.zeros((HD, S), np.float32)
        for p in range(16):
            for j in range(2):
                win[:, 64 * p + 32 * j:64 * p + 32 * j + 32] = \
                    T[:, 2 * p + j:2 * p + j + 32]
        return win.astype(bf16)

    rh_win = windowed(rhT)
    rw_win = windowed(rwT)
    oh = np.zeros((64, S), np.float32)
    for khp in range(H):
        oh[khp, (31 - khp) * W:(31 - khp) * W + W] = 1.0
    for kwp in range(W):
        oh[32 + kwp, 31 - kwp::W] = 1.0
    oh = oh.astype(bf16)
    in_maps = []
    for core in range(NCORES):
        b = core // 2
        h0 = (core % 2) * HPC
        xb = x[b].reshape(S, C)
        xT = np.ascontiguousarray(xb.T).astype(bf16)
        wq = Wqkv[:, h0 * HD:(h0 + HPC) * HD]
        wk = Wqkv[:, C + h0 * HD:C + (h0 + HPC) * HD] * scale
        wqk = np.ascontiguousarray(np.concatenate([wq, wk], axis=1)).astype(bf16)
        wv = np.ascontiguousarray(
            Wqkv[:, 2 * C + h0 * HD:2 * C + (h0 + HPC) * HD]).astype(bf16)
        wp = np.ascontiguousarray(Wproj[h0 * HD:(h0 + HPC) * HD, :]).astype(bf16)
        in_maps.append({"xT": xT, "wqk": wqk, "wv": wv, "wproj": wp,
                        "rh_win": rh_win, "rw_win": rw_win, "onehot": oh})
    return in_maps


_NC_CACHE = {}


def kernel(x, Wqkv, Wproj, bproj, rel_pos_h, rel_pos_w):
    if "nc" not in _NC_CACHE:
        _NC_CACHE["nc"] = build_program()
    nc = _NC_CACHE["nc"]
    in_maps = shard_inputs(x, Wqkv, Wproj, rel_pos_h, rel_pos_w)
    res = run_bass_kernel_spmd(nc, in_maps, list(range(NCORES)))
    bproj = np.asarray(bproj, dtype=np.float32)
    out = np.empty((B, H, W, C), dtype=np.float32)
    for b in range(B):
        acc = res.results[2 * b]["out"] + res.results[2 * b + 1]["out"] + bproj
        out[b] = acc.reshape(H, W, C)
    return out


# revision 12
# speedup vs baseline: 1.4217x; 1.0849x over previous
"""Trainium2 Bass kernel for ViTDet-style attention with decomposed
relative-position bias.

Problem shapes (hardcoded):
  x: (4, 32, 32, 768) f32, Wqkv: (768, 2304), Wproj: (768, 768),
  bproj: (768,), rel_pos_h/w: (63, 64).
  12 heads, head_dim 64, S = 32*32 = 1024.

Sharding: 48 (batch, head) pairs -> 6 heads per core, all of one batch per
core-pair. Each core computes its heads' attention and a partial output
projection (its heads' channel rows of Wproj); the host sums the two
partials per batch and adds bproj.

Device algorithm per core (bf16 matmuls, fp32 PSUM accumulation):
  - qkT = Wqk^T @ x^T  (x^T supplied pre-transposed by host; k pre-scaled)
  - v   = x @ Wv       (natural layout); v_sb per-head segment is
    [v_i (64 cols) | ones (64 cols)] so the av matmul also produces the
    softmax denominator replicated on PSUM partitions 64-127.
  - rel-pos bias computed DIRECTLY in band form (no intermediate table
    product): BhT[r, (h,w)] = sum_c rhT[c, h+r] qT[c, (h,w)] via windowed
    stationaries; two h-values packed per 64x64 matmul (diagonal blocks
    used, off-diagonal garbage ignored).
  - scoresT (k x q) = kaugT^T @ qaugT in ONE K=128 matmul per tile:
    aug rows 0-63 = kT / qT, 64-95 = one-hot(h) / BhT, 96-127 = one-hot(w)/BwT
    => rel-pos bias folded into the QK matmul for free.
  - eT = exp(scoresT) on ScalarE (no max subtraction; scores are O(1)).
  - avT (128 x q): rows 0-63 = out accum, rows 64-127 = denominator.
  - normalize: DVE reciprocal of av[64:128] + DVE multiply. No DMA bounce.
  - partial = out_heads @ Wproj_shard  (natural layout, DMA PSUM->DRAM).
"""

import numpy as np

import concourse.bass as bass
import concourse.bacc as bacc
import concourse.mybir as mybir
import concourse.tile as tile
from concourse.bass_utils import run_bass_kernel_spmd

F32 = mybir.dt.float32
BF16 = mybir.dt.bfloat16

NH = 12          # total heads
C = 768
HD = 64
H = W = 32
S = H * W        # 1024
B = 4
NCORES = 8
HPC = NH * B // NCORES   # heads per core = 6
NCH = 6                  # C // 128 input-channel chunks
NKB = S // 128           # 8 k blocks
NQB = S // 128           # 8 q blocks
NHALF = 512              # matmul moving-dim half


def build_program():
    nc = bacc.Bacc("TRN2", target_bir_lowering=False, debug=False)

    xT = nc.declare_dram_parameter("xT", [C, S], BF16, isOutput=False)
    wqk = nc.declare_dram_parameter("wqk", [C, 2 * HPC * HD], BF16, isOutput=False)
    wv = nc.declare_dram_parameter("wv", [C, HPC * HD], BF16, isOutput=False)
    wproj = nc.declare_dram_parameter("wproj", [HPC * HD, C], BF16, isOutput=False)
    # windowed rel-pos tables: win[:, 64p+32j+r] = T[:, 2p+j+r]
    rh_win = nc.declare_dram_parameter("rh_win", [HD, S], BF16, isOutput=False)
    rw_win = nc.declare_dram_parameter("rw_win", [HD, S], BF16, isOutput=False)
    onehot = nc.declare_dram_parameter("onehot", [64, S], BF16, isOutput=False)
    out = nc.declare_dram_parameter("out", [S, C], F32, isOutput=True)

    with tile.TileContext(nc) as tc:
        with (
            tc.tile_pool(name="persist", bufs=1) as persist,
            tc.tile_pool(name="ps_sc", bufs=2, space="PSUM") as ps_sc,
            tc.tile_pool(name="ps_aux", bufs=2, space="PSUM") as ps_aux,
            tc.tile_pool(name="et", bufs=3) as et_pool,
            tc.tile_pool(name="small", bufs=2) as small,
        ):
            # ---- persistent SBUF loads (interleaved so ci=0 compute can
            # start while later chunks stream in) ----
            xT_sb, wqk_sb, wv_sb = [], [], []
            for ci in range(NCH):
                t = persist.tile([128, S], BF16, tag=f"xT{ci}", name=f"xT{ci}")
                nc.sync.dma_start(t[:], xT[128 * ci:128 * (ci + 1), :])
                xT_sb.append(t)
                t = persist.tile([128, 2 * HPC * HD], BF16, tag=f"wqk{ci}",
                                 name=f"wqk{ci}")
                nc.sync.dma_start(t[:], wqk[128 * ci:128 * (ci + 1), :])
                wqk_sb.append(t)
                t = persist.tile([128, HPC * HD], BF16, tag=f"wv{ci}",
                                 name=f"wv{ci}")
                nc.sync.dma_start(t[:], wv[128 * ci:128 * (ci + 1), :])
                wv_sb.append(t)
            wproj_sb = []
            for ci in range(HPC * HD // 128):
                t = persist.tile([128, C], BF16, tag=f"wproj{ci}", name=f"wproj{ci}")
                nc.sync.dma_start(t[:], wproj[128 * ci:128 * (ci + 1), :])
                wproj_sb.append(t)
            rh_sb = persist.tile([HD, S], BF16, tag="rh", name="rh_sb")
            nc.sync.dma_start(rh_sb[:], rh_win[:, :])
            rw_sb = persist.tile([HD, S], BF16, tag="rw", name="rw_sb")
            nc.sync.dma_start(rw_sb[:], rw_win[:, :])

            # ---- augmented k/q tiles (128, S) per head; one-hot rows DMAed
            # straight from DRAM into kaug rows 64-127 ----
            kaug = [persist.tile([128, S], BF16, tag=f"kaug{i}", name=f"kaug{i}")
                    for i in range(HPC)]
            qaug = [persist.tile([128, S], BF16, tag=f"qaug{i}", name=f"qaug{i}")
                    for i in range(HPC)]
            for i in range(HPC):
                nc.sync.dma_start(kaug[i][64:128, :], onehot[:, :])

            # ---- v tiles: per-head segment [ones (64) | v_i (64)] ----
            # (ones first so the av denominator lands on PSUM partitions 0-63:
            # reciprocal_approx_fast misreads PSUM at base_partition >= 64)
            v_sb = [persist.tile([128, HPC * 2 * HD], BF16, tag=f"v{sb}",
                                 name=f"v{sb}")
                    for sb in range(NKB)]
            for sb in range(NKB):
                ones_dst = bass.AP(v_sb[sb].tensor, v_sb[sb][:].offset,
                                   [v_sb[sb][:].ap[0], [2 * HD, HPC], [1, HD]])
                nc.gpsimd.memset(ones_dst, 1.0)

            # ---- v projection (natural) ----
            for sb in range(NKB):
                vp = ps_aux.tile([128, S], F32, tag="aux", name="vp")
                for ci in range(NCH):
                    nc.tensor.matmul(
                        vp[:, 0:HPC * HD],
                        xT_sb[ci][:, 128 * sb:128 * (sb + 1)],
                        wv_sb[ci][:],
                        start=(ci == 0), stop=(ci == NCH - 1))
                src = bass.AP(vp.tensor, vp[:].offset,
                              [vp[:].ap[0], [HD, HPC], [1, HD]])
                dst = bass.AP(v_sb[sb].tensor, v_sb[sb][:].offset + HD,
                              [v_sb[sb][:].ap[0], [2 * HD, HPC], [1, HD]])
                nc.vector.tensor_copy(dst, src)

            # ---- qk projection (transposed layout) ----
            # qkT octile t covers oc rows [128t, 128t+128): t<3 -> q, t>=3 -> k
            for t in range(2 * HPC * HD // 128):
                qp = ps_aux.tile([128, S], F32, tag="aux", name="qp")
                for ci in range(NCH):
                    for nh in range(S // NHALF):
                        nc.tensor.matmul(
                            qp[:, NHALF * nh:NHALF * (nh + 1)],
                            wqk_sb[ci][:, 128 * t:128 * (t + 1)],
                            xT_sb[ci][:, NHALF * nh:NHALF * (nh + 1)],
                            start=(ci == 0), stop=(ci == NCH - 1))
                for sub in range(2):
                    head = (t % 3) * 2 + sub
                    dst = (qaug if t < 3 else kaug)[head]
                    nc.scalar.copy(dst[0:64, :], qp[64 * sub:64 * sub + 64, :])

            # ---- per head: direct banded rel-pos bias into qaug rows 64-127
            # BhT[r, (h,w)] = sum_c rhT[c, h+r] qT[c, (h,w)]  (r, h in [0,32))
            # Two h-values per matmul: stationary (64, 64) = two overlapping
            # 32-col windows of the table; useful output = diagonal blocks.
            for i in range(HPC):
                bh = ps_sc.tile([64, S], F32, tag="sc", name="bh")
                bw = ps_sc.tile([64, S], F32, tag="sc", name="bw")
                for p in range(16):
                    nc.tensor.matmul(bh[:, 64 * p:64 * (p + 1)],
                                     rh_sb[:, 64 * p:64 * (p + 1)],
                                     qaug[i][0:64, 64 * p:64 * (p + 1)],
                                     start=True, stop=True)
                for p in range(16):
                    rw = bass.AP(qaug[i].tensor, qaug[i][:].offset + 2 * p,
                                 [[S, 64], [1, 2], [W, H]])
                    nc.tensor.matmul(bw[:, 64 * p:64 * (p + 1)],
                                     rw_sb[:, 64 * p:64 * (p + 1)],
                                     rw, start=True, stop=True)
                # copies: diagonal blocks -> qaug rows 64-127 (vector engine)
                pitch = bh[:].ap[0][0]
                # h-axis, j=0: h=2p
                nc.vector.tensor_copy(
                    bass.AP(qaug[i].tensor, qaug[i][:].offset + 64 * S,
                            [[S, 32], [64, 16], [1, 32]]),
                    bass.AP(bh.tensor, bh[:].offset,
                            [[pitch, 32], [64, 16], [1, 32]]))
                # h-axis, j=1: h=2p+1
                nc.vector.tensor_copy(
                    bass.AP(qaug[i].tensor, qaug[i][:].offset + 64 * S + 32,
                            [[S, 32], [64, 16], [1, 32]]),
                    bass.AP(bh.tensor, bh[:].offset + 32 * pitch + 32,
                            [[pitch, 32], [64, 16], [1, 32]]))
                # w-axis, j=0: w=2p, dst col = 32h + w
                nc.vector.tensor_copy(
                    bass.AP(qaug[i].tensor, qaug[i][:].offset + 96 * S,
                            [[S, 32], [2, 16], [W, H]]),
                    bass.AP(bw.tensor, bw[:].offset,
                            [[pitch, 32], [64, 16], [1, 32]]))
                # w-axis, j=1: w=2p+1
                nc.vector.tensor_copy(
                    bass.AP(qaug[i].tensor, qaug[i][:].offset + 96 * S + 1,
                            [[S, 32], [2, 16], [W, H]]),
                    bass.AP(bw.tensor, bw[:].offset + 32 * pitch + 32,
                            [[pitch, 32], [64, 16], [1, 32]]))

            # ---- attention per head ----
            out_headsT = [persist.tile([128, S], BF16, tag=f"ohT{c}",
                                       name=f"ohT{c}")
                          for c in range(HPC * HD // 128)]
            for i in range(HPC):
                av = ps_aux.tile([128, S], F32, tag="aux", name="av")
                for kb in range(NKB):
                    sc = ps_sc.tile([128, S], F32, tag="sc", name="sc")
                    for nh in range(S // NHALF):
                        sl = slice(NHALF * nh, NHALF * (nh + 1))
                        nc.tensor.matmul(
                            sc[:, sl],
                            kaug[i][:, 128 * kb:128 * (kb + 1)],
                            qaug[i][:, sl], start=True, stop=True)
                    e = et_pool.tile([128, S], BF16, tag="et", name="et")
                    nc.scalar.activation(e[:], sc[:],
                                         mybir.ActivationFunctionType.Exp)
                    for nh in range(S // NHALF):
                        sl = slice(NHALF * nh, NHALF * (nh + 1))
                        nc.tensor.matmul(
                            av[:, sl],
                            v_sb[kb][:, 2 * HD * i:2 * HD * (i + 1)],
                            e[:, sl],
                            start=(kb == 0), stop=(kb == NKB - 1))
                rb = small.tile([64, S], F32, tag="rb", name="rb")
                nc.vector.reciprocal_approx_fast(rb[:], av[0:64, :])
                chunk, row = i // 2, (i % 2) * 64
                nc.vector.tensor_tensor(
                    out_headsT[chunk][row:row + 64, :], av[64:128, :], rb[:],
                    op=mybir.AluOpType.mult)

            # ---- output projection (partial) ----
            for qb in range(NQB):
                pp = ps_aux.tile([128, S], F32, tag="aux", name="pp")
                for ci in range(HPC * HD // 128):
                    nc.tensor.matmul(
                        pp[:, 0:NHALF],
                        out_headsT[ci][:, 128 * qb:128 * (qb + 1)],
                        wproj_sb[ci][:, 0:NHALF],
                        start=(ci == 0), stop=(ci == 2))
                    nc.tensor.matmul(
                        pp[:, NHALF:C],
                        out_headsT[ci][:, 128 * qb:128 * (qb + 1)],
                        wproj_sb[ci][:, NHALF:C],
                        start=(ci == 0), stop=(ci == 2))
                pp_sb = small.tile([128, C], F32, tag="pp_sb", name="pp_sb")
                (nc.scalar.copy if qb % 2 else nc.vector.tensor_copy)(
                    pp_sb[:], pp[:, 0:C])
                nc.sync.dma_start(out[128 * qb:128 * (qb + 1), :], pp_sb[:])

    nc.compile()
    return nc


def shard_inputs(x, Wqkv, Wproj, rel_pos_h, rel_pos_w):
    """Build the 8 per-core input maps."""
    import ml_dtypes
    bf16 = ml_dtypes.bfloat16
    scale = HD ** (-0.5)
    x = np.asarray(x, dtype=np.float32)
    Wqkv = np.asarray(Wqkv, dtype=np.float32)
    Wproj = np.asarray(Wproj, dtype=np.float32)
    rhT = np.ascontiguousarray(np.asarray(rel_pos_h, np.float32).T)
    rwT = np.ascontiguousarray(np.asarray(rel_pos_w, np.float32).T)

    def windowed(T):
        win = np.zeros((HD, S), np.float32)
        for p in range(16):
            for j in range(2):
                win[:, 64 * p + 32 * j:64 * p + 32 * j + 32] = \
                    T[:, 2 * p + j:2 * p + j + 32]
        return win.astype(bf16)

    rh_win = windowed(rhT)
    rw_win = windowed(rwT)
    oh = np.zeros((64, S), np.float32)
    for khp in range(H):
        oh[khp, (31 - khp) * W:(31 - khp) * W + W] = 1.0
    for kwp in range(W):
        oh[32 + kwp, 31 - kwp::W] = 1.0
    oh = oh.astype(bf16)
    in_maps = []
    for core in range(NCORES):
        b = core // 2
        h0 = (core % 2) * HPC
        xb = x[b].reshape(S, C)
        xT = np.ascontiguousarray(xb.T).astype(bf16)
        wq = Wqkv[:, h0 * HD:(h0 + HPC) * HD]
        wk = Wqkv[:, C + h0 * HD:C + (h0 + HPC) * HD] * scale
        wqk = np.ascontiguousarray(np.concatenate([wq, wk], axis=1)).astype(bf16)
        wv = np.ascontiguousarray(
            Wqkv[:, 2 * C + h0 * HD:2 * C + (h0 + HPC) * HD]).astype(bf16)
        wp = np.ascontiguousarray(Wproj[h0 * HD:(h0 + HPC) * HD, :]).astype(bf16)
        in_maps.append({"xT": xT, "wqk": wqk, "wv": wv, "wproj": wp,
                        "rh_win": rh_win, "rw_win": rw_win, "onehot": oh})
    return in_maps


_NC_CACHE = {}


def kernel(x, Wqkv, Wproj, bproj, rel_pos_h, rel_pos_w):
    if "nc" not in _NC_CACHE:
        _NC_CACHE["nc"] = build_program()
    nc = _NC_CACHE["nc"]
    in_maps = shard_inputs(x, Wqkv, Wproj, rel_pos_h, rel_pos_w)
    res = run_bass_kernel_spmd(nc, in_maps, list(range(NCORES)))
    bproj = np.asarray(bproj, dtype=np.float32)
    out = np.empty((B, H, W, C), dtype=np.float32)
    for b in range(B):
        acc = res.results[2 * b]["out"] + res.results[2 * b + 1]["out"] + bproj
        out[b] = acc.reshape(H, W, C)
    return out
